# revision 5
# baseline (speedup 1.0000x reference)
"""
Trainium2 Bass kernel for nn_Attention_335007449901 (sparse window attention).

Model (per image, eval mode):
  q = BN(conv1x1(x, wq)); k = BN(conv1x1(x, wk)); v = BN(conv1x1(x, wv))
  7x7 windows over the 112x112 image -> T=256 window tokens, token
  features = (channel, within-window position p) pairs.
  dots[i,j] = <q_i, k_j> * 0.125 ; attn = softmax_j ; out = attn @ v
  y = gelu(out); z = BN(conv1x1(y, wo) + bo); out = gelu(z + x)

Sharding: pure data parallel over batch, 4 images per core on 8 cores.

Implementation notes (v2 - fp8 DoubleRow rework):
  * BN folding as before: M = wk_f^T wq_f (dots via u = M^T x then
    dots_T += u^T x, all bf16), k bias drops, v bias (Bv) applied in the
    first gelu, conv bias + BN fold into the last gelu's bias (Bo).
  * Positions are grouped per window row: a quad (ws2 0-3) and a triple
    (ws2 4-6) per ws1 row -> 14 groups, each with a single uniform
    strided image-layout access pattern.  Fewer, larger ACT/DVE/GPSIMD
    instructions amortize the ~350-cycle per-instruction overhead.
  * The attention-average and output conv run as fp8e4 DoubleRow
    matmuls (2 cols/cycle): v is evacuated as 16*v in fp8 (DVE
    tensor_scalar), attn is normalized into fp8, g = gelu(av/16+Bv) is
    written in fp8, woT is host-scaled by 16 into fp8.  The first gelu
    un-scales via its free `scale=` affine; the residual path scales x
    by 16 through the identity matmul so the last gelu's scale=1/16
    sees wo^T g and x at the same scale.
  * The residual add happens ON THE TENSOR ENGINE: o_ps accumulates
    (16 wo)^T g via a DoubleRow matmul plus (16 I)^T x_win via a plain
    bf16 matmul, so the DVE tensor_tensor add disappears and the final
    gelu reads PSUM directly.
  * PSUM bank budget (8): work ring 3x[128,512] (u conv + v conv),
    dots 1x[128,512], avA/avB 2x[128,1024].  The out conv + residual
    accumulate into the group's kc0 av tile AFTER its gelu read (WAR
    tracked by Tile), so the final gelu reads one contiguous PSUM span
    and no separate out bank is needed.  kc0 alternates avA/avB per
    group so consecutive groups pipeline.
  * dots are computed transposed; softmax normalization via ones-matmul
    reduce; no max subtraction (|dots| < ~30).
  * Final gelu writes in place into the fp32 x image (columns dead
    after the residual matmul read... the residual reads x_winb bf16,
    x_img columns are dead after xwin build + store staging reuse).
"""

import numpy as np

IN_C = 128
HIDE_C = 256
HC2 = 128
OUT_C = 128
WS = 7
SCALE = 0.125
EPS = 1e-5
B, H, W = 32, 112, 112
HW = H * W          # 12544
H1 = H // WS        # 16
W1 = W // WS        # 16
T = H1 * W1         # 256 windows
NP = WS * WS        # 49 positions
NCORES = 8
BPC = B // NCORES   # images per core

F32 = np.float32


def _groups():
    """Row groups: per window-row ws1 a quad (ws2 0..3) and a triple
    (ws2 4..6).  Each group is one uniform strided AP in image layout.
    Returns (ws1, ws2_start, cnt, col_base) with col_base the column
    offset in the position-raster window layout x_winb."""
    out = []
    for ws1 in range(WS):
        out.append((ws1, 0, 4, (ws1 * WS + 0) * T))
        out.append((ws1, 4, 3, (ws1 * WS + 4) * T))
    return out


def build_bass_kernel(bpc=BPC):
    import concourse.bass as bass
    import concourse.tile as tile
    import concourse.mybir as mybir
    from concourse import bacc

    f32 = mybir.dt.float32
    bf16 = mybir.dt.bfloat16
    fp8 = mybir.dt.float8e4
    AF = mybir.ActivationFunctionType
    DR = mybir.MatmulPerfMode.DoubleRow

    nc = bacc.Bacc("TRN2", target_bir_lowering=False)

    x_d = nc.dram_tensor("x", [bpc, IN_C, HW], f32, kind="ExternalInput")
    m_d = nc.dram_tensor("m", [IN_C, IN_C], bf16, kind="ExternalInput")
    h_d = nc.dram_tensor("hcol", [IN_C, 1], bf16, kind="ExternalInput")
    wvT_d = nc.dram_tensor("wvT", [IN_C, HIDE_C], bf16, kind="ExternalInput")
    # 16*wo_f^T in fp8, kc-split: [128 d-half, 2 kc, 128 o]
    woT_d = nc.dram_tensor("woT", [HIDE_C, OUT_C], fp8, kind="ExternalInput")
    # 16*I in bf16 for the residual matmul
    eye_d = nc.dram_tensor("eye16", [IN_C, IN_C], bf16, kind="ExternalInput")
    # packed per-partition fp32 bias columns: [Bv_lo, Bv_hi, Bo]
    bias_d = nc.dram_tensor("biases", [128, 3], f32, kind="ExternalInput")
    out_d = nc.dram_tensor("out", [bpc, OUT_C, HW], f32, kind="ExternalOutput")

    groups = _groups()

    with tile.TileContext(nc) as tc:
        with (
            tc.tile_pool(name="singles", bufs=1) as singles,
            tc.tile_pool(name="xpool", bufs=2) as xpool,
            tc.tile_pool(name="xwin", bufs=2) as xwin_pool,
            tc.tile_pool(name="u_sb", bufs=4) as u_sb_pool,
            tc.tile_pool(name="v_sb", bufs=14) as v_sb_pool,
            tc.tile_pool(name="g_sb", bufs=2) as g_sb_pool,
            tc.tile_pool(name="attn_sb", bufs=2) as attn_pool,
            tc.tile_pool(name="small_sb", bufs=2) as small_pool,
            tc.tile_pool(name="ps_work", bufs=3, space="PSUM") as ps_work,
            tc.tile_pool(name="ps_dots", bufs=1, space="PSUM") as ps_dots,
            tc.tile_pool(name="ps_av", bufs=1, space="PSUM") as ps_av,
        ):
            # ---- weights / constants (resident) ----
            m_sb = singles.tile([128, IN_C], bf16)
            nc.sync.dma_start(out=m_sb, in_=m_d.ap())
            h_sb = singles.tile([128, 1], bf16)
            nc.sync.dma_start(out=h_sb, in_=h_d.ap())
            wvT = singles.tile([128, HIDE_C], bf16)
            nc.sync.dma_start(out=wvT, in_=wvT_d.ap())
            woT = singles.tile([128, 2, OUT_C], fp8)
            nc.sync.dma_start(
                out=woT, in_=woT_d.ap().rearrange("(kc p) m -> p kc m", kc=2)
            )
            eye16 = singles.tile([128, IN_C], bf16)
            nc.sync.dma_start(out=eye16, in_=eye_d.ap())
            biases = singles.tile([128, 3], f32)
            nc.sync.dma_start(out=biases, in_=bias_d.ap())
            bv_ap = [biases[:, 0:1], biases[:, 1:2]]
            bo_ap = biases[:, 2:3]

            # 1/64 so s_ps = s/64 and r = 64/s: attn is carried as 64*attn
            # in fp8 to stay out of the subnormal range (diffuse attention
            # weights are ~1/256); the first gelu un-scales via scale=1/1024
            # (64 from attn, 16 from v).
            ones_mat = singles.tile([128, 128], bf16)
            nc.vector.memset(ones_mat, 1.0 / 64.0)

            for img in range(bpc):
                # ---- load x image ----
                x_img = xpool.tile([128, HW], f32, tag="ximg")
                for dc in range(4):
                    nc.sync.dma_start(
                        out=x_img[:, dc * (HW // 4):(dc + 1) * (HW // 4)],
                        in_=x_d.ap()[img, :, dc * (HW // 4):(dc + 1) * (HW // 4)])
                # dummy exp so walrus places the exp ACT-table load here,
                # off the softmax critical chain
                scratch = small_pool.tile([128, 1], f32, tag="scratch")
                nc.scalar.activation(scratch, biases[:, 0:1], AF.Exp)
                x5 = x_img.rearrange("p (h a w b) -> p h a w b", h=H1, a=WS, b=WS)

                def grp_ap(gr):
                    """strided image-layout AP of this group's positions,
                    position-major to match x_winb columns"""
                    ws1, b0, cnt, _ = gr
                    return x5[:, :, ws1, :, b0:b0 + cnt].rearrange(
                        "p h w b -> p b h w")

                # ---- build position-raster bf16 window copy ----
                x_winb = xwin_pool.tile([128, NP * T], bf16, tag="xwin")
                for gi, gr in enumerate(groups):
                    _, _, cnt, base = gr
                    dst = x_winb[:, base:base + cnt * T]
                    # split the window permute across the three copy engines
                    if gi % 7 in (0, 2, 4):          # 6 groups -> gpsimd
                        nc.gpsimd.tensor_copy(dst, grp_ap(gr))
                    elif gi % 7 in (1, 5):           # 4 groups -> scalar
                        nc.scalar.activation(dst, grp_ap(gr), AF.Copy, scale=1.0)
                    else:                            # 4 groups -> vector
                        nc.vector.tensor_copy(dst, grp_ap(gr))

                # ---- phase 1: dots_T accumulation over positions ----
                dots_t = ps_dots.tile([128, 512], f32, tag="dots", name="dots")
                dots = [dots_t[:, 0:T], dots_t[:, T:2 * T]]
                chunk_starts = list(range(0, NP, 2))   # 2 positions per chunk
                nchunks = len(chunk_starts)

                def u_conv(ci, p0):
                    npos = min(2, NP - p0)
                    N = npos * T
                    base = p0 * T
                    u_ps = ps_work.tile([128, 512], f32, tag="pwork")
                    nc.tensor.matmul(u_ps[:, :N], lhsT=m_sb,
                                     rhs=x_winb[:, base:base + N],
                                     start=True, stop=True)
                    u_sbt = u_sb_pool.tile([128, 512], bf16, tag="u")
                    if ci % 2 == 0:
                        nc.scalar.activation(u_sbt[:, :N], u_ps[:, :N],
                                             AF.Copy, scale=1.0)
                    else:
                        nc.vector.tensor_copy(u_sbt[:, :N], u_ps[:, :N])
                    return u_sbt

                def dots_mms(ci, p0, u_sbt):
                    npos = min(2, NP - p0)
                    base = p0 * T
                    first = ci == 0
                    for pi in range(npos):
                        for jh in (0, 1):
                            nc.tensor.matmul(
                                dots[jh],
                                lhsT=u_sbt[:, pi * T + jh * 128:
                                           pi * T + jh * 128 + 128],
                                rhs=x_winb[:, base + pi * T:
                                           base + (pi + 1) * T],
                                start=first and pi == 0 and jh == 0,
                                stop=False,
                                skip_group_check=True)

                pend = []
                for ci, p0 in enumerate(chunk_starts):
                    u_sbt = u_conv(ci, p0)
                    if len(pend) >= 2:
                        dots_mms(*pend.pop(0))
                    pend.append((ci, p0, u_sbt))
                for pe_ in pend:
                    dots_mms(*pe_)
                # c[j] = sum_p h . x_p[:, j] as one fixed-weight run (h stays
                # loaded), then added into dots via two rank-1 matmuls.
                # c_row borrows a work PSUM slot.
                c_ps = ps_work.tile([128, 512], f32, tag="pwork")
                c_row_ps = c_ps[0:1, 0:T]
                for p in range(NP):
                    nc.tensor.matmul(c_row_ps, lhsT=h_sb,
                                     rhs=x_winb[:, p * T:(p + 1) * T],
                                     start=p == 0, stop=p == NP - 1)
                c_row = small_pool.tile([1, T], bf16, tag="csb")
                nc.vector.tensor_copy(c_row, c_row_ps)
                ones_r = small_pool.tile([1, T], bf16, tag="ones_r")
                nc.vector.memset(ones_r, 1.0)
                for jh in (0, 1):
                    nc.tensor.matmul(
                        dots[jh], lhsT=c_row[:, jh * 128:jh * 128 + 128],
                        rhs=ones_r, start=False, stop=jh == 1,
                        skip_group_check=True)

                # ---- early v-convs (overlap the softmax chain on PE) ----
                def v_conv(gr):
                    """per position: v_ps [tok(jc), (jc,d)] -> 16*v in fp8"""
                    ws1, b0, cnt, base = gr
                    vsb = []
                    for pi in range(cnt):
                        v_ps = ps_work.tile([128, 512], f32, tag="pwork")
                        for jc in (0, 1):
                            nc.tensor.matmul(
                                v_ps[:, jc * HIDE_C:(jc + 1) * HIDE_C],
                                lhsT=x_winb[:, base + pi * T + jc * 128:
                                            base + pi * T + jc * 128 + 128],
                                rhs=wvT,
                                start=True, stop=True)
                        v_sbt = v_sb_pool.tile([128, 512], fp8, tag="v")
                        nc.vector.tensor_scalar_mul(v_sbt, v_ps, 16.0)
                        vsb.append(v_sbt)
                    return vsb

                LA = 2
                vcache = {gi: v_conv(groups[gi]) for gi in range(LA)}

                # ---- softmax over j (= partitions of dots_T) ----
                e_sb = attn_pool.tile([128, 512], bf16, tag="esb", name="esb")
                nc.scalar.activation(e_sb, dots_t, AF.Exp)
                s_ps = ps_dots.tile([128, T], f32, tag="dots", name="ssum")
                for jc in (0, 1):
                    nc.tensor.matmul(s_ps, lhsT=ones_mat,
                                     rhs=e_sb[:, jc * T:(jc + 1) * T],
                                     start=jc == 0, stop=jc == 1)
                r_sb = small_pool.tile([128, T], f32, tag="rsb")
                nc.vector.reciprocal(r_sb, s_ps)
                attn = attn_pool.tile([128, 512], fp8, tag="attn", name="attn")
                for jc in (0, 1):
                    nc.vector.tensor_mul(attn[:, jc * T:(jc + 1) * T],
                                         e_sb[:, jc * T:(jc + 1) * T], r_sb)
                attn3 = attn.rearrange("p (jc i) -> p jc i", jc=2)

                # ---- phase 2: attention-average, out-conv, residual ----
                for gi, gr in enumerate(groups):
                    ws1, b0, cnt, base = gr
                    N = cnt * T
                    if gi + LA < len(groups):
                        vcache[gi + LA] = v_conv(groups[gi + LA])
                    vsb = vcache.pop(gi)

                    # kc0 home alternates avA/avB so consecutive groups
                    # pipeline; out+residual accumulate into the kc0 tile
                    # after its gelu read.
                    tA = ps_av.tile([128, 1024], f32, tag="avA", name="avA")
                    tB = ps_av.tile([128, 1024], f32, tag="avB", name="avB")
                    kc_tile = (tA, tB) if gi % 2 == 0 else (tB, tA)

                    g_t = g_sb_pool.tile([128, 2048], fp8, tag="g")
                    for kc in (0, 1):
                        av = kc_tile[kc]
                        for pi in range(cnt):
                            v3 = vsb[pi].rearrange("p (jc d) -> p jc d", jc=2)
                            nc.tensor.matmul(
                                av[:, pi * T:(pi + 1) * T],
                                lhsT=v3[:, :, kc * 128:kc * 128 + 128],
                                rhs=attn3,
                                start=True, stop=True,
                                perf_mode=DR)
                        # g = gelu(av/1024 + Bv[kc]); fp8 out for the DR conv
                        nc.scalar.activation(
                            g_t[:, kc * 1024:kc * 1024 + N], av[:, :N],
                            AF.Gelu, bias=bv_ap[kc], scale=1.0 / 1024.0)

                    # out conv (DoubleRow over kc) + residual (16*I @ x_win),
                    # accumulated into the kc0 av tile (free after its gelu)
                    g3 = g_t.rearrange("p (kc n) -> p kc n", kc=2)
                    o_ps = kc_tile[0]
                    for h0 in range(0, N, 512):
                        n = min(512, N - h0)
                        nc.tensor.matmul(
                            o_ps[:, h0:h0 + n],
                            lhsT=woT,
                            rhs=g3[:, :, h0:h0 + n],
                            start=True, stop=False,
                            perf_mode=DR,
                            skip_group_check=True)
                        nc.tensor.matmul(
                            o_ps[:, h0:h0 + n],
                            lhsT=eye16,
                            rhs=x_winb[:, base + h0:base + h0 + n],
                            start=False, stop=True,
                            skip_group_check=True)
                    # final gelu((o + 16x)/16 + Bo), written strided into
                    # the (dead) x image columns for the output DMA
                    nc.scalar.activation(grp_ap(gr), o_ps[:, :N], AF.Gelu,
                                         bias=bo_ap, scale=1.0 / 16.0)

                # ---- store (x_img now holds the output image) ----
                for dc in range(4):
                    nc.sync.dma_start(
                        out=out_d.ap()[img, :, dc * (HW // 4):(dc + 1) * (HW // 4)],
                        in_=x_img[:, dc * (HW // 4):(dc + 1) * (HW // 4)])

    nc.compile()
    return nc


def fold_params(wq, gq, bq, mq, vq, wk, gk, bk, mk, vk,
                wv, gv, bv, mv, vv, wo, bo, go, bbo, mo, vo):
    """Host-side BN/bias folding. Returns (M, h, wvT, woT_fp8, eye16, biases)."""
    import ml_dtypes
    bf16 = ml_dtypes.bfloat16
    f8 = ml_dtypes.float8_e4m3

    aq = gq / np.sqrt(vq + EPS)
    wq_f = (SCALE * aq)[:, None] * wq
    Bq = SCALE * (bq - aq * mq)

    ak = gk / np.sqrt(vk + EPS)
    wk_f = ak[:, None] * wk          # k bias drops (softmax shift invariance)

    M = wk_f.T @ wq_f                # dots_T = sum_p (M^T x_p)^T x_p
    hv = wk_f.T @ Bq                 # c[j] = sum_p hv . x_p[:, j]

    av = gv / np.sqrt(vv + EPS)
    wv_f = av[:, None] * wv
    Bv = bv - av * mv                # applied inside the first gelu

    ao = go / np.sqrt(vo + EPS)
    wo_f = ao[:, None] * wo
    Bo = ao * (bo - mo) + bbo        # conv bias + BN fold, inside last gelu

    biases = np.stack([Bv[:128], Bv[128:], Bo], axis=1).astype(F32)
    woT16 = np.clip(16.0 * wo_f.T, -240.0, 240.0)
    eye16 = 16.0 * np.eye(IN_C, dtype=np.float64)
    return (np.ascontiguousarray(M).astype(bf16),
            np.ascontiguousarray(hv[:, None]).astype(bf16),
            np.ascontiguousarray(wv_f.T).astype(bf16),
            np.ascontiguousarray(woT16).astype(f8),
            np.ascontiguousarray(eye16).astype(bf16),
            biases)


_CACHED = {}


def _get_nc(bpc=BPC):
    if bpc not in _CACHED:
        _CACHED[bpc] = build_bass_kernel(bpc)
    return _CACHED[bpc]


def make_in_maps(inputs):
    x = np.asarray(inputs["x"], F32)
    m, hv, wvT, woT, eye16, biases = fold_params(
        *[np.asarray(inputs[k], F32) for k in
          ("wq", "gq", "bq", "mq", "vq", "wk", "gk", "bk", "mk", "vk",
           "wv", "gv", "bv", "mv", "vv", "wo", "bo", "go", "bbo", "mo", "vo")]
    )
    in_maps = []
    for c in range(NCORES):
        xs = np.ascontiguousarray(
            x[c * BPC:(c + 1) * BPC].reshape(BPC, IN_C, HW))
        in_maps.append({"x": xs, "m": m, "hcol": hv, "wvT": wvT,
                        "woT": woT, "eye16": eye16, "biases": biases})
    return in_maps


def kernel(**inputs):
    from concourse.bass_utils import run_bass_kernel_spmd

    in_maps = make_in_maps(inputs)
    nc = _get_nc(BPC)
    res = run_bass_kernel_spmd(nc, in_maps, list(range(NCORES)))
    outs = [res.results[c]["out"].reshape(BPC, OUT_C, H, W)
            for c in range(NCORES)]
    return np.concatenate(outs, axis=0)


# revision 6
# speedup vs baseline: 1.0520x; 1.0520x over previous
"""
Trainium2 Bass kernel for nn_Attention_335007449901 (sparse window attention).

Model (per image, eval mode):
  q = BN(conv1x1(x, wq)); k = BN(conv1x1(x, wk)); v = BN(conv1x1(x, wv))
  7x7 windows over the 112x112 image -> T=256 window tokens, token
  features = (channel, within-window position p) pairs.
  dots[i,j] = <q_i, k_j> * 0.125 ; attn = softmax_j ; out = attn @ v
  y = gelu(out); z = BN(conv1x1(y, wo) + bo); out = gelu(z + x)

Sharding: pure data parallel over batch, 4 images per core on 8 cores.

Implementation notes (v3 - software-pipelined emission):
  * BN folding: M = wk_f^T wq_f (dots via u = M^T x then dots_T += u^T x,
    bf16), k bias drops, v bias (Bv) in the first gelu, conv bias + BN in
    the last gelu's bias (Bo).
  * Positions grouped per window row: quad (ws2 0-3) + triple (ws2 4-6)
    per ws1 row -> 14 groups, each one uniform strided AP.
  * Attention-average and output conv are fp8e4 DoubleRow matmuls: v
    evacuated as 16*v fp8, attn carried as 64*attn fp8 (1/64 in the
    softmax-sum ones matrix keeps diffuse weights out of the subnormal
    range), g = gelu(av/1024+Bv) in fp8, woT host-scaled by 16 to fp8.
  * Residual add on the TensorEngine: o_ps accumulates (16 wo)^T g via
    DoubleRow plus (16 I)^T x_win bf16; the final gelu reads PSUM with
    scale=1/16 and writes the x image in place.  Its input AP is
    reordered token-major so the strided image write has 16B-contiguous
    runs (4 adjacent pixels) instead of isolated 4B elements.
  * PSUM (8 banks): work ring 3x[128,512] (u + v convs), dots 1x[128,512],
    avA/avB 2x[128,1024].  Out conv + residual accumulate into the
    group's kc0 av tile after its gelu read (WAR tracked by Tile); kc0
    alternates avA/avB per group so consecutive groups pipeline.
  * EMISSION IS SOFTWARE-PIPELINED ACROSS IMAGES: image i+1's DMA load
    is emitted before image i's phase 2, and its window-permute copies +
    u-conv/dots chunks are interleaved into image i's 14 phase-2 group
    slots.  Without this, the in-order engine queues leave the PE idle
    during the ACT-heavy phase 2 and vice versa (~150us/core of bubbles).
"""

import numpy as np

IN_C = 128
HIDE_C = 256
HC2 = 128
OUT_C = 128
WS = 7
SCALE = 0.125
EPS = 1e-5
B, H, W = 32, 112, 112
HW = H * W          # 12544
H1 = H // WS        # 16
W1 = W // WS        # 16
T = H1 * W1         # 256 windows
NP = WS * WS        # 49 positions
NCORES = 8
BPC = B // NCORES   # images per core

F32 = np.float32


def _groups():
    """Row groups: per window-row ws1 a quad (ws2 0..3) and a triple
    (ws2 4..6).  (ws1, ws2_start, cnt, col_base) with col_base the
    column offset in the position-raster window layout x_winb."""
    out = []
    for ws1 in range(WS):
        out.append((ws1, 0, 4, (ws1 * WS + 0) * T))
        out.append((ws1, 4, 3, (ws1 * WS + 4) * T))
    return out


GROUPS = _groups()
NG = len(GROUPS)     # 14
NCHUNK = (NP + 1) // 2  # 25 u-conv chunks of 2 positions
LA = 2               # v-conv lookahead groups inside phase 2
DOTS_LAG = 2         # u chunks in flight before dots matmuls start


def _positions_done(nglots):
    """window positions covered by the first n xwin groups"""
    return (nglots + 1) // 2 * 4 + nglots // 2 * 3


def build_bass_kernel(bpc=BPC):
    import concourse.bass as bass
    import concourse.tile as tile
    import concourse.mybir as mybir
    from concourse import bacc

    f32 = mybir.dt.float32
    bf16 = mybir.dt.bfloat16
    fp8 = mybir.dt.float8e4
    AF = mybir.ActivationFunctionType
    DR = mybir.MatmulPerfMode.DoubleRow

    nc = bacc.Bacc("TRN2", target_bir_lowering=False)

    x_d = nc.dram_tensor("x", [bpc, IN_C, HW], f32, kind="ExternalInput")
    m_d = nc.dram_tensor("m", [IN_C, IN_C], bf16, kind="ExternalInput")
    h_d = nc.dram_tensor("hcol", [IN_C, 1], bf16, kind="ExternalInput")
    wvT_d = nc.dram_tensor("wvT", [IN_C, HIDE_C], bf16, kind="ExternalInput")
    woT_d = nc.dram_tensor("woT", [HIDE_C, OUT_C], fp8, kind="ExternalInput")
    eye_d = nc.dram_tensor("eye16", [IN_C, IN_C], bf16, kind="ExternalInput")
    bias_d = nc.dram_tensor("biases", [128, 3], f32, kind="ExternalInput")
    out_d = nc.dram_tensor("out", [bpc, OUT_C, HW], f32, kind="ExternalOutput")

    with tile.TileContext(nc) as tc:
        with (
            tc.tile_pool(name="singles", bufs=1) as singles,
            tc.tile_pool(name="xpool", bufs=2) as xpool,
            tc.tile_pool(name="xwin", bufs=2) as xwin_pool,
            tc.tile_pool(name="u_sb", bufs=5) as u_sb_pool,
            tc.tile_pool(name="v_sb", bufs=14) as v_sb_pool,
            tc.tile_pool(name="g_sb", bufs=2) as g_sb_pool,
            tc.tile_pool(name="attn_sb", bufs=2) as attn_pool,
            tc.tile_pool(name="small_sb", bufs=2) as small_pool,
            tc.tile_pool(name="ps_work", bufs=3, space="PSUM") as ps_work,
            tc.tile_pool(name="ps_dots", bufs=1, space="PSUM") as ps_dots,
            tc.tile_pool(name="ps_av", bufs=1, space="PSUM") as ps_av,
        ):
            # ---- weights / constants (resident) ----
            m_sb = singles.tile([128, IN_C], bf16)
            nc.sync.dma_start(out=m_sb, in_=m_d.ap())
            h_sb = singles.tile([128, 1], bf16)
            nc.sync.dma_start(out=h_sb, in_=h_d.ap())
            wvT = singles.tile([128, HIDE_C], bf16)
            nc.sync.dma_start(out=wvT, in_=wvT_d.ap())
            woT = singles.tile([128, 2, OUT_C], fp8)
            nc.sync.dma_start(
                out=woT, in_=woT_d.ap().rearrange("(kc p) m -> p kc m", kc=2)
            )
            eye16 = singles.tile([128, IN_C], bf16)
            nc.sync.dma_start(out=eye16, in_=eye_d.ap())
            biases = singles.tile([128, 3], f32)
            nc.sync.dma_start(out=biases, in_=bias_d.ap())
            bv_ap = [biases[:, 0:1], biases[:, 1:2]]
            bo_ap = biases[:, 2:3]

            # 1/64 so s_ps = s/64 and r = 64/s: attn is carried as 64*attn
            ones_mat = singles.tile([128, 128], bf16)
            nc.vector.memset(ones_mat, 1.0 / 64.0)
            ones_r = singles.tile([1, T], bf16)
            nc.vector.memset(ones_r, 1.0)

            class _Img:
                pass

            def img_new(img):
                S = _Img()
                S.img = img
                S.pend = []
                S.nchunks_emitted = 0
                S.vcache = {}
                return S

            def emit_load(S):
                S.x_img = xpool.tile([128, HW], f32, tag="ximg")
                for dc in range(4):
                    nc.sync.dma_start(
                        out=S.x_img[:, dc * (HW // 4):(dc + 1) * (HW // 4)],
                        in_=x_d.ap()[S.img, :,
                                     dc * (HW // 4):(dc + 1) * (HW // 4)])
                S.x5 = S.x_img.rearrange("p (h a w b) -> p h a w b",
                                         h=H1, a=WS, b=WS)
                S.x_winb = xwin_pool.tile([128, NP * T], bf16, tag="xwin")

            def grp_ap(S, gr):
                """strided image-layout AP, position-major (matches x_winb)"""
                ws1, b0, cnt, _ = gr
                return S.x5[:, :, ws1, :, b0:b0 + cnt].rearrange(
                    "p h w b -> p b h w")

            def emit_xwin(S, gi):
                gr = GROUPS[gi]
                _, _, cnt, base = gr
                dst = S.x_winb[:, base:base + cnt * T]
                if gi % 7 in (0, 2, 4):          # 6 groups -> gpsimd
                    nc.gpsimd.tensor_copy(dst, grp_ap(S, gr))
                elif gi % 7 == 5:                # 2 groups -> scalar
                    nc.scalar.activation(dst, grp_ap(S, gr), AF.Copy,
                                         scale=1.0)
                else:                            # 6 groups -> vector
                    nc.vector.tensor_copy(dst, grp_ap(S, gr))

            def _dots_mms(S, ent):
                ci, p0, u_sbt = ent
                npos = min(2, NP - p0)
                base = p0 * T
                for pi in range(npos):
                    for jh in (0, 1):
                        nc.tensor.matmul(
                            S.dots[jh],
                            lhsT=u_sbt[:, pi * T + jh * 128:
                                       pi * T + jh * 128 + 128],
                            rhs=S.x_winb[:, base + pi * T:
                                         base + (pi + 1) * T],
                            start=(ci == 0 and pi == 0 and jh == 0),
                            stop=False,
                            skip_group_check=True)

            def emit_chunk(S):
                ci = S.nchunks_emitted
                S.nchunks_emitted += 1
                p0 = 2 * ci
                if ci == 0:
                    S.dots_t = ps_dots.tile([128, 512], f32, tag="dots",
                                            name="dots")
                    S.dots = [S.dots_t[:, 0:T], S.dots_t[:, T:2 * T]]
                npos = min(2, NP - p0)
                N = npos * T
                base = p0 * T
                u_ps = ps_work.tile([128, 512], f32, tag="pwork")
                nc.tensor.matmul(u_ps[:, :N], lhsT=m_sb,
                                 rhs=S.x_winb[:, base:base + N],
                                 start=True, stop=True)
                u_sbt = u_sb_pool.tile([128, 512], bf16, tag="u")
                if ci % 3 == 0:
                    nc.scalar.activation(u_sbt[:, :N], u_ps[:, :N],
                                         AF.Copy, scale=1.0)
                else:
                    nc.vector.tensor_copy(u_sbt[:, :N], u_ps[:, :N])
                if len(S.pend) >= DOTS_LAG:
                    _dots_mms(S, S.pend.pop(0))
                S.pend.append((ci, p0, u_sbt))

            def emit_softmax(S):
                for ent in S.pend:
                    _dots_mms(S, ent)
                S.pend = []
                # c[j] = sum_p h . x_p[:, j]: one fixed-weight run, then
                # two rank-1 matmuls into dots
                c_ps = ps_work.tile([128, 512], f32, tag="pwork")
                c_row_ps = c_ps[0:1, 0:T]
                for p in range(NP):
                    nc.tensor.matmul(c_row_ps, lhsT=h_sb,
                                     rhs=S.x_winb[:, p * T:(p + 1) * T],
                                     start=p == 0, stop=p == NP - 1)
                c_row = small_pool.tile([1, T], bf16, tag="csb")
                nc.vector.tensor_copy(c_row, c_row_ps)
                for jh in (0, 1):
                    nc.tensor.matmul(
                        S.dots[jh], lhsT=c_row[:, jh * 128:jh * 128 + 128],
                        rhs=ones_r, start=False, stop=jh == 1,
                        skip_group_check=True)
                # softmax over j (partitions of dots_T)
                e_sb = attn_pool.tile([128, 512], bf16, tag="esb", name="esb")
                nc.scalar.activation(e_sb, S.dots_t, AF.Exp)
                s_ps = ps_dots.tile([128, T], f32, tag="dots", name="ssum")
                for jc in (0, 1):
                    nc.tensor.matmul(s_ps, lhsT=ones_mat,
                                     rhs=e_sb[:, jc * T:(jc + 1) * T],
                                     start=jc == 0, stop=jc == 1)
                r_sb = small_pool.tile([128, T], f32, tag="rsb")
                nc.vector.reciprocal(r_sb, s_ps)
                attn = attn_pool.tile([128, 512], fp8, tag="attn",
                                      name="attn")
                for jc in (0, 1):
                    nc.vector.tensor_mul(attn[:, jc * T:(jc + 1) * T],
                                         e_sb[:, jc * T:(jc + 1) * T], r_sb)
                S.attn3 = attn.rearrange("p (jc i) -> p jc i", jc=2)

            def v_conv(S, gi):
                """per position: v_ps [tok(jc), (jc,d)] -> 16*v in fp8"""
                ws1, b0, cnt, base = GROUPS[gi]
                vsb = []
                for pi in range(cnt):
                    v_ps = ps_work.tile([128, 512], f32, tag="pwork")
                    for jc in (0, 1):
                        nc.tensor.matmul(
                            v_ps[:, jc * HIDE_C:(jc + 1) * HIDE_C],
                            lhsT=S.x_winb[:, base + pi * T + jc * 128:
                                          base + pi * T + jc * 128 + 128],
                            rhs=wvT,
                            start=True, stop=True)
                    v_sbt = v_sb_pool.tile([128, 512], fp8, tag="v")
                    nc.vector.tensor_scalar_mul(v_sbt, v_ps, 16.0)
                    vsb.append(v_sbt)
                return vsb

            def emit_early_v(S):
                for gi in range(LA):
                    S.vcache[gi] = v_conv(S, gi)

            def emit_p2_group(S, gi):
                gr = GROUPS[gi]
                ws1, b0, cnt, base = gr
                N = cnt * T
                if gi + LA < NG:
                    S.vcache[gi + LA] = v_conv(S, gi + LA)
                vsb = S.vcache.pop(gi)

                tA = ps_av.tile([128, 1024], f32, tag="avA", name="avA")
                tB = ps_av.tile([128, 1024], f32, tag="avB", name="avB")
                kc_tile = (tA, tB) if gi % 2 == 0 else (tB, tA)

                g_t = g_sb_pool.tile([128, 2048], fp8, tag="g")
                for kc in (0, 1):
                    av = kc_tile[kc]
                    for pi in range(cnt):
                        v3 = vsb[pi].rearrange("p (jc d) -> p jc d", jc=2)
                        nc.tensor.matmul(
                            av[:, pi * T:(pi + 1) * T],
                            lhsT=v3[:, :, kc * 128:kc * 128 + 128],
                            rhs=S.attn3,
                            start=True, stop=True,
                            perf_mode=DR)
                    # g = gelu(av/1024 + Bv[kc]); fp8 out for the DR conv
                    nc.scalar.activation(
                        g_t[:, kc * 1024:kc * 1024 + N], av[:, :N],
                        AF.Gelu, bias=bv_ap[kc], scale=1.0 / 1024.0)

                # out conv (DoubleRow over kc) + residual (16*I @ x_win),
                # accumulated into the kc0 av tile (free after its gelu)
                g3 = g_t.rearrange("p (kc n) -> p kc n", kc=2)
                o_ps = kc_tile[0]
                for h0 in range(0, N, 512):
                    n = min(512, N - h0)
                    nc.tensor.matmul(
                        o_ps[:, h0:h0 + n],
                        lhsT=woT,
                        rhs=g3[:, :, h0:h0 + n],
                        start=True, stop=False,
                        perf_mode=DR,
                        skip_group_check=True)
                    nc.tensor.matmul(
                        o_ps[:, h0:h0 + n],
                        lhsT=eye16,
                        rhs=S.x_winb[:, base + h0:base + h0 + n],
                        start=False, stop=True,
                        skip_group_check=True)
                # final gelu((o + 16x)/16 + Bo), written in place into the
                # (dead) x image columns.  Token-major order on both sides
                # gives the strided image write 4-pixel contiguous runs.
                nc.scalar.activation(
                    S.x5[:, :, ws1, :, b0:b0 + cnt],
                    o_ps[:, :N].rearrange("p (b hw) -> p hw b", b=cnt),
                    AF.Gelu, bias=bo_ap, scale=1.0 / 16.0)

            def emit_store(S):
                for dc in range(4):
                    nc.sync.dma_start(
                        out=out_d.ap()[S.img, :,
                                       dc * (HW // 4):(dc + 1) * (HW // 4)],
                        in_=S.x_img[:, dc * (HW // 4):(dc + 1) * (HW // 4)])

            # ---- software-pipelined schedule ----
            S = img_new(0)
            emit_load(S)
            for gi in range(NG):
                emit_xwin(S, gi)
            while S.nchunks_emitted < NCHUNK:
                emit_chunk(S)
            emit_softmax(S)
            emit_early_v(S)

            for img in range(bpc):
                nx = None
                if img + 1 < bpc:
                    nx = img_new(img + 1)
                    emit_load(nx)
                for s in range(NG):
                    emit_p2_group(S, s)
                    if nx is not None:
                        emit_xwin(nx, s)
                        avail = _positions_done(s)
                        while (nx.nchunks_emitted < NCHUNK and
                               min(2 * nx.nchunks_emitted + 2, NP) <= avail):
                            emit_chunk(nx)
                if nx is not None:
                    while nx.nchunks_emitted < NCHUNK:
                        emit_chunk(nx)
                    emit_softmax(nx)
                    emit_early_v(nx)
                emit_store(S)
                S = nx

    nc.compile()
    return nc


def fold_params(wq, gq, bq, mq, vq, wk, gk, bk, mk, vk,
                wv, gv, bv, mv, vv, wo, bo, go, bbo, mo, vo):
    """Host-side BN/bias folding. Returns (M, h, wvT, woT_fp8, eye16, biases)."""
    import ml_dtypes
    bf16 = ml_dtypes.bfloat16
    f8 = ml_dtypes.float8_e4m3

    aq = gq / np.sqrt(vq + EPS)
    wq_f = (SCALE * aq)[:, None] * wq
    Bq = SCALE * (bq - aq * mq)

    ak = gk / np.sqrt(vk + EPS)
    wk_f = ak[:, None] * wk          # k bias drops (softmax shift invariance)

    M = wk_f.T @ wq_f                # dots_T = sum_p (M^T x_p)^T x_p
    hv = wk_f.T @ Bq                 # c[j] = sum_p hv . x_p[:, j]

    av = gv / np.sqrt(vv + EPS)
    wv_f = av[:, None] * wv
    Bv = bv - av * mv                # applied inside the first gelu

    ao = go / np.sqrt(vo + EPS)
    wo_f = ao[:, None] * wo
    Bo = ao * (bo - mo) + bbo        # conv bias + BN fold, inside last gelu

    biases = np.stack([Bv[:128], Bv[128:], Bo], axis=1).astype(F32)
    woT16 = np.clip(16.0 * wo_f.T, -240.0, 240.0)
    eye16 = 16.0 * np.eye(IN_C, dtype=np.float64)
    return (np.ascontiguousarray(M).astype(bf16),
            np.ascontiguousarray(hv[:, None]).astype(bf16),
            np.ascontiguousarray(wv_f.T).astype(bf16),
            np.ascontiguousarray(woT16).astype(f8),
            np.ascontiguousarray(eye16).astype(bf16),
            biases)


_CACHED = {}


def _get_nc(bpc=BPC):
    if bpc not in _CACHED:
        _CACHED[bpc] = build_bass_kernel(bpc)
    return _CACHED[bpc]


def make_in_maps(inputs):
    x = np.asarray(inputs["x"], F32)
    m, hv, wvT, woT, eye16, biases = fold_params(
        *[np.asarray(inputs[k], F32) for k in
          ("wq", "gq", "bq", "mq", "vq", "wk", "gk", "bk", "mk", "vk",
           "wv", "gv", "bv", "mv", "vv", "wo", "bo", "go", "bbo", "mo", "vo")]
    )
    in_maps = []
    for c in range(NCORES):
        xs = np.ascontiguousarray(
            x[c * BPC:(c + 1) * BPC].reshape(BPC, IN_C, HW))
        in_maps.append({"x": xs, "m": m, "hcol": hv, "wvT": wvT,
                        "woT": woT, "eye16": eye16, "biases": biases})
    return in_maps


def kernel(**inputs):
    from concourse.bass_utils import run_bass_kernel_spmd

    in_maps = make_in_maps(inputs)
    nc = _get_nc(BPC)
    res = run_bass_kernel_spmd(nc, in_maps, list(range(NCORES)))
    outs = [res.results[c]["out"].reshape(BPC, OUT_C, H, W)
            for c in range(NCORES)]
    return np.concatenate(outs, axis=0)
